# revision 1
# baseline (speedup 1.0000x reference)
"""Distributed Trainium2 (8 NeuronCores) attention-head kernel.

Problem: single attention head with projections.
  q = Q @ Wq.T + bq ; k = K @ Wk.T + bk ; v = V @ Wv.T + bv
  x = (q @ k.T) / sqrt(64) ; x = x*m - 1e9*(1-m) ; p = softmax(x)
  y = p @ v
Shapes: Q/K/V [2, 4096, 1024] f32, mask [2, 4096, 4096] int32 -> y [2, 4096, 64] f32.

Strategy (8 cores): shard queries 8-ways (2 batches x 4 query-chunks of 1024
rows).  K/V are replicated within each 4-core batch group (collective_compute
has ~100us fixed overhead on this fleet; bf16 replication is cheaper).  The
host reshards into matmul-native transposed layouts (contraction dim on
partitions), packed partition-major so every DMA is a full-width [128, W] tile
with >=8KB contiguous per-partition rows (measured ~430GB/s vs 41GB/s at 1KB
rows).  K/V are additionally packed s-group-major so each 2MB group load is
immediately projectable (the dm-contraction needs all 8 dm-chunks of a group).
Q/K/V/W are cast to bf16, the 0/1 mask to fp8e4 (exact); softmax is computed
as p=exp(x/8)*m, y=(p@v)/sum(p) - algebraically identical to the reference's
masked softmax (no fully-masked rows exist).

Per-core pipeline (layouts [partitions, free]):
  qT[64,1024] = sum_j WqT[j].T @ QT[j]        (dm-chunk j, PSUM accumulate)
  per s-group g (4 x 1024): kT[:,g] = proj(K), vT[:,g] = proj(V),
     v_aug[s,65] = [v | 1] via PE transposes of vT
     per s-chunk j (8 x 128): sT = kT_chunk.T @ qT ; p = exp(sT/8) * maskT
                              yT[65,1024] += v_aug_chunk.T @ p  (PSUM accum)
  y[q,65] = transpose(yT); out = y[:, :64] / y[:, 64:65]
DMA issue order == consumption order (per-engine FIFO queues preserve it).
"""

import numpy as np
import ml_dtypes

import concourse.bass as bass
import concourse.mybir as mybir
import concourse.tile as tile
from concourse import bacc
import concourse.bass_utils as bass_utils
from concourse.bass_utils import run_bass_kernel_spmd
from concourse.masks import make_identity

B, S, DM, DK = 2, 4096, 1024, 64
N_CORES = 8
GROUP = 4            # cores per batch
SQ = S // GROUP      # query rows per core (1024)
NDM = DM // 128      # dm chunks (8)
NSG = 4              # s groups (1024 rows each)
SG = S // NSG        # 1024
NSC_G = SG // 128    # s chunks per group (8)

F32 = mybir.dt.float32
BF16 = mybir.dt.bfloat16
FP8 = mybir.dt.float8e4

_last_results = None


def _build():
    nc = bacc.Bacc(None, target_bir_lowering=False)

    qt_e = nc.declare_dram_parameter("qt", [128, NDM * SQ], BF16, isOutput=False)
    kt_e = nc.declare_dram_parameter("kt", [128, NDM * S], BF16, isOutput=False)
    vt_e = nc.declare_dram_parameter("vt", [128, NDM * S], BF16, isOutput=False)
    mt_e = nc.declare_dram_parameter("mt", [128, S * SQ // 128], FP8, isOutput=False)
    w_e = nc.declare_dram_parameter("wqkv", [128, 3 * NDM * DK], BF16, isOutput=False)
    b_e = nc.declare_dram_parameter("bqkv", [DK, 3], F32, isOutput=False)
    out_e = nc.declare_dram_parameter("out", [128, (SQ // 128) * DK], F32, isOutput=True)

    GW = NDM * SG    # columns per kt/vt group slice (8192)
    MW = NSC_G * SQ  # columns per mask group slice (8192)

    with tile.TileContext(nc) as tc:
        with (
            tc.tile_pool(name="const", bufs=1) as cpool,
            tc.tile_pool(name="mask", bufs=NSG) as mpool,
            tc.tile_pool(name="kin", bufs=3) as kpool,
            tc.tile_pool(name="big", bufs=1) as bigpool,
            tc.tile_pool(name="pp", bufs=6) as ppool,
            tc.tile_pool(name="small", bufs=4) as spool,
            tc.tile_pool(name="psum", bufs=1, space="PSUM") as pproj,
            tc.tile_pool(name="psum_s", bufs=2, space="PSUM") as psT,
            tc.tile_pool(name="psum_y", bufs=1, space="PSUM") as pyT,
        ):
            # ---- loads, in consumption order ----
            qt_sb = cpool.tile([128, NDM * SQ], BF16, tag="qt")
            nc.sync.dma_start(qt_sb[:], qt_e[:])
            w_sb = cpool.tile([128, 3 * NDM * DK], BF16, tag="w")
            nc.sync.dma_start(w_sb[:], w_e[:])
            b_sb = cpool.tile([DK, 3], F32, tag="b")
            nc.sync.dma_start(b_sb[:], b_e[:])

            kt_t, vt_t, mq_t = {}, {}, {}
            for g in range(NSG):
                kt_t[g] = kpool.tile([128, GW], BF16, tag="xin", name=f"kt_g{g}")
                nc.sync.dma_start(kt_t[g][:], kt_e[:, g * GW:(g + 1) * GW])
                if g == 0:
                    mq_t[0] = mpool.tile([128, MW], FP8, tag="mt", name="mq_g0")
                    nc.sync.dma_start(mq_t[0][:], mt_e[:, 0:MW])
                vt_t[g] = kpool.tile([128, GW], BF16, tag="xin", name=f"vt_g{g}")
                nc.sync.dma_start(vt_t[g][:], vt_e[:, g * GW:(g + 1) * GW])
                if g in (1, 2):
                    mq_t[g] = mpool.tile([128, MW], FP8, tag="mt", name=f"mq_g{g}")
                    nc.sync.dma_start(mq_t[g][:], mt_e[:, g * MW:(g + 1) * MW])
            mq_t[3] = mpool.tile([128, MW], FP8, tag="mt", name="mq_g3")
            nc.sync.dma_start(mq_t[3][:], mt_e[:, 3 * MW:4 * MW])

            ident_bf = cpool.tile([128, 128], BF16, tag="ident_bf")
            make_identity(nc, ident_bf[:])
            ident_f32 = cpool.tile([128, 128], F32, tag="ident_f32")
            make_identity(nc, ident_f32[:])

            def wsl(which, j):  # weight chunk slice in w_sb
                return w_sb[:, (which * NDM + j) * DK:(which * NDM + j + 1) * DK]

            # ---- q projection: qT[64, 1024] ----
            qT_sb = bigpool.tile([DK, SQ], BF16, tag="qT")
            ps = pproj.tile([DK, 1024], F32, tag="proj")
            for j in range(NDM):
                for h in range(2):
                    c0 = j * SQ + h * 512
                    nc.tensor.matmul(
                        ps[:, h * 512:(h + 1) * 512],
                        lhsT=wsl(0, j), rhs=qt_sb[:, c0:c0 + 512],
                        start=(j == 0), stop=(j == NDM - 1),
                    )
            nc.vector.tensor_scalar_add(qT_sb[:], ps[:], b_sb[:, 0:1])

            kT_sb = bigpool.tile([DK, S], BF16, tag="kT")
            vT_sb = bigpool.tile([DK, S], BF16, tag="vT")
            v_aug = bigpool.tile([128, S // 128 * 65], BF16, tag="vaug")
            nc.vector.memset(v_aug[:], 1.0)
            yT_ps = pyT.tile([65, SQ], F32, tag="yT")

            for g in range(NSG):
                # ---- k/v projections for this s-group ----
                for which, t, dst in ((1, kt_t[g], kT_sb), (2, vt_t[g], vT_sb)):
                    ps = pproj.tile([DK, 1024], F32, tag="proj", name=f"ps_{which}_{g}")
                    for j in range(NDM):
                        for h in range(2):
                            c0 = j * SG + h * 512
                            nc.tensor.matmul(
                                ps[:, h * 512:(h + 1) * 512],
                                lhsT=wsl(which, j), rhs=t[:, c0:c0 + 512],
                                start=(j == 0), stop=(j == NDM - 1),
                            )
                    nc.vector.tensor_scalar_add(
                        dst[:, g * SG:(g + 1) * SG], ps[:], b_sb[:, which:which + 1]
                    )
                # ---- v_aug transposes for this group ----
                for jj in range(NSC_G):
                    sc = g * NSC_G + jj
                    pt = psT.tile([128, DK], BF16, tag="sT", name=f"pt_{sc}")
                    nc.tensor.transpose(
                        pt[:], vT_sb[:, sc * 128:(sc + 1) * 128],
                        ident_bf[:DK, :DK],
                    )
                    nc.vector.tensor_copy(v_aug[:, sc * 65:sc * 65 + DK], pt[:])
                # ---- main loop chunks of this group ----
                for jj in range(NSC_G):
                    sc = g * NSC_G + jj
                    sT = psT.tile([128, SQ], F32, tag="sT", name=f"sT_{sc}")
                    for h in range(2):
                        nc.tensor.matmul(
                            sT[:, h * 512:(h + 1) * 512],
                            lhsT=kT_sb[:, sc * 128:(sc + 1) * 128],
                            rhs=qT_sb[:, h * 512:(h + 1) * 512],
                            start=True, stop=True,
                        )
                    p = ppool.tile([128, SQ], BF16, tag="p", name=f"p_{sc}")
                    nc.scalar.activation(
                        p[:], sT[:], mybir.ActivationFunctionType.Exp, scale=0.125
                    )
                    nc.vector.tensor_mul(
                        p[:], p[:], mq_t[g][:, jj * SQ:(jj + 1) * SQ]
                    )
                    for h in range(2):
                        nc.tensor.matmul(
                            yT_ps[:, h * 512:(h + 1) * 512],
                            lhsT=v_aug[:, sc * 65:(sc + 1) * 65],
                            rhs=p[:, h * 512:(h + 1) * 512],
                            start=(sc == 0), stop=(sc == S // 128 - 1),
                        )

            # ---- epilogue: y = transpose(yT); out = y[:, :64] / y[:, 64] ----
            yT_sb = bigpool.tile([65, SQ], F32, tag="yT_sb")
            nc.scalar.copy(yT_sb[:], yT_ps[:])
            y_all = bigpool.tile([128, (SQ // 128) * DK], F32, tag="y_all")
            for t in range(SQ // 128):
                yp = psT.tile([128, 65], F32, tag="sT", name=f"yp_{t}")
                nc.tensor.transpose(
                    yp[:], yT_sb[:, t * 128:(t + 1) * 128], ident_f32[:65, :65]
                )
                rcp = spool.tile([128, 1], F32, tag="rcp", name=f"rcp_{t}")
                nc.vector.reciprocal(rcp[:], yp[:, DK:DK + 1])
                nc.vector.tensor_scalar_mul(
                    y_all[:, t * DK:(t + 1) * DK], yp[:, :DK], rcp[:]
                )
            nc.sync.dma_start(out_e[:], y_all[:])

    nc.finalize()
    return nc


def _pack(at, w):
    """[R, W] -> [128, (R//128)*W]: row p gets rows {p, 128+p, ...}."""
    r = at.shape[0]
    return np.ascontiguousarray(
        at.reshape(r // 128, 128, w).transpose(1, 0, 2).reshape(128, -1)
    )


def _pack_groups(at):
    """KT/VT [1024, 4096] -> [128, 4*8*1024]: s-group-major partition packing.
    col ((g*8 + j)*1024 + s') on row p = at[j*128 + p, g*1024 + s']."""
    a = at.reshape(NDM, 128, NSG, SG)         # [j, p, g, s']
    return np.ascontiguousarray(
        a.transpose(1, 2, 0, 3).reshape(128, -1)
    )


def kernel(Q, K, V, mask, Wq, bq, Wk, bk, Wv, bv):
    global _last_results
    bf16 = ml_dtypes.bfloat16
    fp8 = ml_dtypes.float8_e4m3

    w_p = np.concatenate(
        [_pack(W.T.astype(bf16), DK) for W in (Wq, Wk, Wv)], axis=1
    )
    b_p = np.ascontiguousarray(
        np.stack([bq, bk, bv], axis=1).astype(np.float32)
    )

    kt_b = [_pack_groups(K[b].T.astype(bf16)) for b in range(B)]
    vt_b = [_pack_groups(V[b].T.astype(bf16)) for b in range(B)]

    in_maps = []
    for c in range(N_CORES):
        b, i = divmod(c, GROUP)
        rows = slice(i * SQ, (i + 1) * SQ)
        in_maps.append({
            "qt": _pack(np.ascontiguousarray(Q[b, rows, :].T).astype(bf16), SQ),
            "kt": kt_b[b],
            "vt": vt_b[b],
            "mt": _pack(np.ascontiguousarray(mask[b, rows, :].T).astype(fp8), SQ),
            "wqkv": w_p,
            "bqkv": b_p,
        })

    nc = _build()
    res = run_bass_kernel_spmd(nc, in_maps, core_ids=list(range(N_CORES)))
    _last_results = res

    out = np.empty((B, S, DK), dtype=np.float32)
    for c in range(N_CORES):
        b, i = divmod(c, GROUP)
        y = res.results[c]["out"].reshape(128, SQ // 128, DK).transpose(1, 0, 2)
        out[b, i * SQ:(i + 1) * SQ, :] = y.reshape(SQ, DK)
    return out



# revision 13
# speedup vs baseline: 1.4338x; 1.4338x over previous
"""Distributed Trainium2 (8 NeuronCores) attention-head kernel, v2.

Problem: single attention head with projections.
  q = Q @ Wq.T + bq ; k = K @ Wk.T + bk ; v = V @ Wv.T + bv
  x = (q @ k.T) / 8 ; x = x*m - 1e9*(1-m) ; p = softmax(x) ; y = p @ v
Shapes: Q/K/V [2, 4096, 1024] f32, mask [2, 4096, 4096] int32 -> y [2, 4096, 64].

Sharding (8 cores): 2x2 grid per batch (flash-decoding style per the hint):
core (b, qh, kh) handles 2048 queries x 2048 keys and returns UNNORMALIZED
partial stats yT[65, 2048] = [sum_s p_s v_s ; sum_s p_s]; the host combines
the two kh partials per (b, qh): y = (yA+yB)[:64] / (yA+yB)[64].  This is the
"all-gathered softmax statistics" combine done at unshard time (collectives
on this fleet cost ~100us fixed, host combine is ~2M flops).

Device pipeline (all matmuls bf16, psum f32):
  - projections col-tiled (out width 64 -> two 64-row col strips run
    concurrently in the PE array); qT is produced duplicated on both
    partition halves, kT split even/odd chunk so scores can row-tile.
  - scores: contraction is only dk=64, so 4 (K=64, M=64) tiles run
    concurrently via tile_position row+col strips (~2x).
  - mask: folded into the scores PSUM by an fp8 DoubleRow identity matmul
    (psum += 240*m), then ACT computes p = exp(0.125*s + 30m - 30) in one
    pass - the masked softmax exactly (leak exp(-30+6) ~ 4e-11, negligible).
    No DVE/Pool elementwise mask work, mask DMA stays 1 byte/elem.
  - y: yT[65, :] += v_aug.T @ p accumulated over key chunks (v_aug has a
    ones column -> row 64 = sum p).
  - PE warmup matmuls at t=0 engage the HAM clock gate (1.2 -> 2.4 GHz).
"""

import numpy as np
import ml_dtypes

import concourse.bass as bass
import concourse.mybir as mybir
import concourse.tile as tile
from concourse import bacc
from concourse.bass_utils import run_bass_kernel_spmd
from concourse.masks import make_identity

B, S, DM, DK = 2, 4096, 1024, 64
N_CORES = 8
SQ = 2048            # queries per core
SK = 2048            # keys per core
NG = 8               # key groups per core (256 keys each)
NJ = DM // 128       # dm chunks (8)

F32 = mybir.dt.float32
BF16 = mybir.dt.bfloat16
FP8 = mybir.dt.float8e4
DR = mybir.MatmulPerfMode.DoubleRow
EXP = mybir.ActivationFunctionType.Exp

MASK_W = 240.0       # ident weight: exp(0.125*(s + 240*m) - 30) = exp(s/8 + 30m - 30)
N_WARM = 20          # PE warmup matmuls
DEBUG = False        # add intermediate dumps

_last_results = None


def _build():
    nc = bacc.Bacc(None, target_bir_lowering=False)

    qt_e = nc.declare_dram_parameter("qt", [128, 2 * NJ * 1024], BF16, isOutput=False)
    kt_e = nc.declare_dram_parameter("kt", [128, NG * NJ * 256], BF16, isOutput=False)
    vt_e = nc.declare_dram_parameter("vt", [128, NG * NJ * 256], BF16, isOutput=False)
    m0_e = nc.declare_dram_parameter("m0", [128, 16, 1024], FP8, isOutput=False)
    m1_e = nc.declare_dram_parameter("m1", [128, 16, 1024], FP8, isOutput=False)
    w_e = nc.declare_dram_parameter("wqkv", [128, 3 * NJ * DK], BF16, isOutput=False)
    b_e = nc.declare_dram_parameter("bqkv", [128, 3], F32, isOutput=False)
    id_e = nc.declare_dram_parameter("identdr", [128, 2, 256], FP8, isOutput=False)
    out_e = nc.declare_dram_parameter("out", [65, SQ], F32, isOutput=True)
    if DEBUG:
        dbg_e = {
            "d_qT0": nc.declare_dram_parameter("d_qT0", [128, 1024], BF16, isOutput=True),
            "d_kT": nc.declare_dram_parameter("d_kT", [128, NG * 128], BF16, isOutput=True),
            "d_vaug": nc.declare_dram_parameter("d_vaug", [128, 16 * 65], BF16, isOutput=True),
            "d_p": nc.declare_dram_parameter("d_p", [128, 1024], BF16, isOutput=True),
        }

    with tile.TileContext(nc) as tc:
        with (
            tc.tile_pool(name="const", bufs=1) as cpool,
            tc.tile_pool(name="inp", bufs=1) as ipool,
            tc.tile_pool(name="work", bufs=1) as spool,
            tc.tile_pool(name="pp", bufs=3) as ppool,
            tc.tile_pool(name="ps_work", bufs=2, space="PSUM") as pwork,
            tc.tile_pool(name="ps_y", bufs=1, space="PSUM") as py,
            tc.tile_pool(name="ps_kv", bufs=1, space="PSUM") as pkv,
        ):
            # ---- constants / warmup (no DMA deps) ----
            wu = cpool.tile([128, 512], BF16, tag="wu")
            nc.vector.memset(wu[:], 0.0)
            nbias = cpool.tile([128, 1], F32, tag="nbias")
            nc.vector.memset(nbias[:], -30.0)
            act_w = spool.tile([128, 32], BF16, tag="actw")
            nc.scalar.activation(act_w[:], wu[:, 0:32], EXP, bias=nbias[:])  # pull exp tables early
            ident_bf = cpool.tile([128, 128], BF16, tag="identbf")
            make_identity(nc, ident_bf[:])

            wups = pwork.tile([128, 1024], F32, tag="sAB", name="wups")
            for i in range(N_WARM):
                nc.tensor.matmul(
                    wups[:, 0:512], lhsT=wu[:, 0:128], rhs=wu[:],
                    start=True, stop=True, skip_group_check=True,
                )

            # ---- input DMAs (issue order ~= arrival order) ----
            qt_sb = ipool.tile([128, 2 * NJ * 1024], BF16, tag="qt")
            nc.sync.dma_start(qt_sb[:, 0:8192], qt_e[:, 0:8192])
            w_sb = cpool.tile([128, 3 * NJ * DK], BF16, tag="w")
            nc.sync.dma_start(w_sb[:], w_e[:])
            b_sb = cpool.tile([128, 3], F32, tag="b")
            nc.sync.dma_start(b_sb[:], b_e[:])
            id_sb = cpool.tile([128, 2, 256], FP8, tag="ident")
            nc.sync.dma_start(id_sb[:], id_e[:])

            kt_sb = ipool.tile([128, NG * 2048], BF16, tag="kt")
            vt_sb = ipool.tile([128, NG * 2048], BF16, tag="vt")
            m0_sb = ipool.tile([128, 16, 1024], FP8, tag="m0")
            m1_sb = ipool.tile([128, 16, 1024], FP8, tag="m1")
            for g in range(NG):
                cs = slice(g * 2048, (g + 1) * 2048)
                nc.sync.dma_start(kt_sb[:, cs], kt_e[:, cs])
                nc.sync.dma_start(vt_sb[:, cs], vt_e[:, cs])
                nc.sync.dma_start(m0_sb[:, 2 * g:2 * g + 2, :], m0_e[:, 2 * g:2 * g + 2, :])
            nc.sync.dma_start(qt_sb[:, 8192:16384], qt_e[:, 8192:16384])
            nc.sync.dma_start(m1_sb[:], m1_e[:])

            def wsl(which, j):
                return w_sb[:, (which * NJ + j) * DK:(which * NJ + j + 1) * DK]

            # ---- persistent work tiles ----
            qT = {}
            kT = spool.tile([128, NG * 128], BF16, tag="kT")
            vT = spool.tile([64, NG * 256], BF16, tag="vT")
            v_aug = spool.tile([128, 16 * 65], BF16, tag="vaug")
            nc.vector.memset(v_aug[:], 1.0)

            def q_proj(h):
                qps = pwork.tile([128, 1024], F32, tag="sAB", name=f"qps{h}")
                for j in range(NJ):
                    for s in range(2):
                        rhs = qt_sb[:, h * 8192 + j * 1024 + s * 512:
                                    h * 8192 + j * 1024 + (s + 1) * 512]
                        for st in range(2):
                            nc.tensor.matmul(
                                qps[st * 64:(st + 1) * 64, s * 512:(s + 1) * 512],
                                lhsT=wsl(0, j), rhs=rhs,
                                start=(j == 0), stop=(j == NJ - 1),
                            )
                qT[h] = spool.tile([128, 1024], BF16, tag=f"qT{h}", name=f"qT{h}")
                nc.vector.tensor_scalar_add(qT[h][:], qps[:], b_sb[:, 0:1])

            def kv_proj(g):
                kps = pkv.tile([128, 128], F32, tag="kps", name=f"kps{g}")
                for j in range(NJ):
                    c0 = g * 2048 + j * 256
                    nc.tensor.matmul(
                        kps[0:64, :], lhsT=wsl(1, j), rhs=kt_sb[:, c0:c0 + 128],
                        start=(j == 0), stop=(j == NJ - 1),
                    )
                    nc.tensor.matmul(
                        kps[64:128, :], lhsT=wsl(1, j), rhs=kt_sb[:, c0 + 128:c0 + 256],
                        start=(j == 0), stop=(j == NJ - 1),
                    )
                nc.vector.tensor_scalar_add(
                    kT[:, g * 128:(g + 1) * 128], kps[:], b_sb[:, 1:2]
                )
                vps = pkv.tile([64, 256], F32, tag="vps", name=f"vps{g}")
                for j in range(NJ):
                    c0 = g * 2048 + j * 256
                    nc.tensor.matmul(
                        vps[:], lhsT=wsl(2, j), rhs=vt_sb[:, c0:c0 + 256],
                        start=(j == 0), stop=(j == NJ - 1),
                    )
                nc.vector.tensor_scalar_add(
                    vT[:, g * 256:(g + 1) * 256], vps[:], b_sb[0:64, 2:3]
                )
                for c in range(2):
                    vtr = pkv.tile([128, 64], BF16, tag="kps", name=f"vtr{g}_{c}")
                    nc.tensor.transpose(
                        vtr[:], vT[0:64, g * 256 + c * 128:g * 256 + (c + 1) * 128],
                        ident_bf[0:64, 0:64],
                    )
                    nc.vector.tensor_copy(
                        v_aug[:, (2 * g + c) * 65:(2 * g + c) * 65 + 64], vtr[:]
                    )

            def main_step(g, h, s, y_ps, m_sb):
                sAB = pwork.tile([128, 1024], F32, tag="sAB", name=f"s{h}_{g}_{s}")
                qc = slice(s * 512, (s + 1) * 512)
                kc = g * 128
                # scores: 4 concurrent (K=64, M=64) tiles
                nc.tensor.matmul(
                    sAB[0:64, 0:512], lhsT=kT[0:64, kc:kc + 64],
                    rhs=qT[h][0:64, qc], start=True, stop=False,
                    skip_group_check=True,
                )
                nc.tensor.matmul(
                    sAB[64:128, 0:512], lhsT=kT[0:64, kc + 64:kc + 128],
                    rhs=qT[h][0:64, qc], start=True, stop=False,
                    skip_group_check=True,
                )
                nc.tensor.matmul(
                    sAB[0:64, 512:1024], lhsT=kT[64:128, kc:kc + 64],
                    rhs=qT[h][64:128, qc], start=True, stop=False,
                    skip_group_check=True,
                )
                nc.tensor.matmul(
                    sAB[64:128, 512:1024], lhsT=kT[64:128, kc + 64:kc + 128],
                    rhs=qT[h][64:128, qc], start=True, stop=False,
                    skip_group_check=True,
                )
                # mask add: psum += 240*m via fp8 DoubleRow identity
                m_rhs = m_sb[:, 2 * g:2 * g + 2, s * 512:(s + 1) * 512]
                nc.tensor.matmul(
                    sAB[:, 0:512], lhsT=id_sb[:, :, 0:128], rhs=m_rhs,
                    start=False, stop=True, perf_mode=DR, skip_group_check=True,
                )
                nc.tensor.matmul(
                    sAB[:, 512:1024], lhsT=id_sb[:, :, 128:256], rhs=m_rhs,
                    start=False, stop=True, perf_mode=DR, skip_group_check=True,
                )
                p = ppool.tile([128, 1024], BF16, tag="p", name=f"p{h}_{g}_{s}")
                nc.scalar.activation(p[:], sAB[:], EXP, bias=nbias[:], scale=0.125)
                if DEBUG and (g, h, s) == (0, 0, 0):
                    nc.sync.dma_start(dbg_e["d_p"][:], p[:])
                nc.tensor.matmul(
                    y_ps[:, qc], lhsT=v_aug[:, (2 * g) * 65:(2 * g) * 65 + 65],
                    rhs=p[:, 0:512], start=(g == 0), stop=False,
                    skip_group_check=True,
                )
                nc.tensor.matmul(
                    y_ps[:, qc], lhsT=v_aug[:, (2 * g + 1) * 65:(2 * g + 1) * 65 + 65],
                    rhs=p[:, 512:1024], start=False, stop=(g == NG - 1),
                    skip_group_check=True,
                )

            # ---- pass 0 (q half 0) with per-group projections ----
            q_proj(0)
            y0 = py.tile([65, 1024], F32, tag="y", name="y0")
            for g in range(NG):
                kv_proj(g)
                for s in range(2):
                    main_step(g, 0, s, y0, m0_sb)
            ysb0 = spool.tile([65, 1024], F32, tag="ysb0")
            nc.vector.tensor_copy(ysb0[:], y0[:])
            nc.sync.dma_start(out_e[:, 0:1024], ysb0[:])

            # ---- pass 1 (q half 1) ----
            q_proj(1)
            y1 = py.tile([65, 1024], F32, tag="y", name="y1")
            for g in range(NG):
                for s in range(2):
                    main_step(g, 1, s, y1, m1_sb)
            ysb1 = spool.tile([65, 1024], F32, tag="ysb1")
            nc.vector.tensor_copy(ysb1[:], y1[:])
            nc.sync.dma_start(out_e[:, 1024:2048], ysb1[:])

            if DEBUG:
                nc.sync.dma_start(dbg_e["d_qT0"][:], qT[0][:])
                nc.sync.dma_start(dbg_e["d_kT"][:], kT[:])
                nc.sync.dma_start(dbg_e["d_vaug"][:], v_aug[:])

    nc.finalize()
    return nc


def _pack_x(x):
    """[2048 rows, 1024 dm] f32 -> qt layout [128, 2*8*1024] (h, j, q')."""
    t = x.T.reshape(NJ, 128, 2, 1024)          # [j, p, h, q']
    return np.ascontiguousarray(
        t.transpose(1, 2, 0, 3).reshape(128, -1)
    ).astype(ml_dtypes.bfloat16)


def _pack_kv(x):
    """[2048 keys, 1024 dm] f32 -> [128, 8*8*256] (g, j, r)."""
    t = x.T.reshape(NJ, 128, NG, 256)          # [j, p, g, r]
    return np.ascontiguousarray(
        t.transpose(1, 2, 0, 3).reshape(128, -1)
    ).astype(ml_dtypes.bfloat16)


def _pack_mask(mblk):
    """mask block [2048 q, 2048 k] int -> (m0, m1) each [128, 16, 1024] fp8.
    element (key = g*256 + j*128 + p, q = h*1024 + q') at m{h}[p, 2g+j, q']."""
    t = mblk.T.reshape(NG, 2, 128, 2, 1024)    # [g, j, p, h, q']
    t = t.transpose(2, 3, 0, 1, 4)             # [p, h, g, j, q']
    m = np.ascontiguousarray(t.reshape(128, 2, 16, 1024)).astype(ml_dtypes.float8_e4m3)
    return m[:, 0], m[:, 1]


def kernel(Q, K, V, mask, Wq, bq, Wk, bk, Wv, bv):
    global _last_results
    bf16 = ml_dtypes.bfloat16
    fp8 = ml_dtypes.float8_e4m3

    Q, K, V = (np.asarray(a, dtype=np.float32) for a in (Q, K, V))
    mask = np.asarray(mask)

    w_p = np.concatenate(
        [np.ascontiguousarray(
            W.T.reshape(NJ, 128, DK).transpose(1, 0, 2).reshape(128, NJ * DK)
         ).astype(bf16) for W in (Wq, Wk, Wv)],
        axis=1,
    )
    b_p = np.ascontiguousarray(
        np.stack([np.tile(np.asarray(b, np.float32), 2) for b in (bq, bk, bv)], axis=1)
    )
    ident = np.zeros((128, 2, 2, 128), dtype=np.float32)
    for p in range(128):
        ident[p, 0, 0, p] = MASK_W
        ident[p, 1, 1, p] = MASK_W
    ident = ident.reshape(128, 2, 256).astype(fp8)

    qt_c = {(b, qh): _pack_x(Q[b, qh * SQ:(qh + 1) * SQ]) for b in range(B) for qh in range(2)}
    kt_c = {(b, kh): _pack_kv(K[b, kh * SK:(kh + 1) * SK]) for b in range(B) for kh in range(2)}
    vt_c = {(b, kh): _pack_kv(V[b, kh * SK:(kh + 1) * SK]) for b in range(B) for kh in range(2)}

    in_maps = []
    for c in range(N_CORES):
        b, r = divmod(c, 4)
        qh, kh = divmod(r, 2)
        m0, m1 = _pack_mask(mask[b, qh * SQ:(qh + 1) * SQ, kh * SK:(kh + 1) * SK])
        in_maps.append({
            "qt": qt_c[(b, qh)], "kt": kt_c[(b, kh)], "vt": vt_c[(b, kh)],
            "m0": m0, "m1": m1,
            "wqkv": w_p, "bqkv": b_p, "identdr": ident,
        })

    nc = _build()
    res = run_bass_kernel_spmd(nc, in_maps, core_ids=list(range(N_CORES)))
    _last_results = res

    out = np.empty((B, S, DK), dtype=np.float32)
    for b in range(B):
        for qh in range(2):
            yA = res.results[b * 4 + qh * 2 + 0]["out"].astype(np.float64)
            yB = res.results[b * 4 + qh * 2 + 1]["out"].astype(np.float64)
            ysum = yA + yB
            y = ysum[:DK] / ysum[DK:DK + 1]
            out[b, qh * SQ:(qh + 1) * SQ, :] = y.T.astype(np.float32)
    return out


# revision 14
# speedup vs baseline: 1.4347x; 1.0007x over previous
"""Distributed Trainium2 (8 NeuronCores) attention-head kernel, v2.

Problem: single attention head with projections.
  q = Q @ Wq.T + bq ; k = K @ Wk.T + bk ; v = V @ Wv.T + bv
  x = (q @ k.T) / 8 ; x = x*m - 1e9*(1-m) ; p = softmax(x) ; y = p @ v
Shapes: Q/K/V [2, 4096, 1024] f32, mask [2, 4096, 4096] int32 -> y [2, 4096, 64].

Sharding (8 cores): 2x2 grid per batch (flash-decoding style per the hint):
core (b, qh, kh) handles 2048 queries x 2048 keys and returns UNNORMALIZED
partial stats yT[65, 2048] = [sum_s p_s v_s ; sum_s p_s]; the host combines
the two kh partials per (b, qh): y = (yA+yB)[:64] / (yA+yB)[64].  This is the
"all-gathered softmax statistics" combine done at unshard time (collectives
on this fleet cost ~100us fixed, host combine is ~2M flops).

Device pipeline (all matmuls bf16, psum f32):
  - projections col-tiled (out width 64 -> two 64-row col strips run
    concurrently in the PE array); qT is produced duplicated on both
    partition halves, kT split even/odd chunk so scores can row-tile.
  - scores: contraction is only dk=64, so 4 (K=64, M=64) tiles run
    concurrently via tile_position row+col strips (~2x).
  - mask: folded into the scores PSUM by an fp8 DoubleRow identity matmul
    (psum += 240*m), then ACT computes p = exp(0.125*s + 30m - 30) in one
    pass - the masked softmax exactly (leak exp(-30+6) ~ 4e-11, negligible).
    No DVE/Pool elementwise mask work, mask DMA stays 1 byte/elem.
  - y: yT[65, :] += v_aug.T @ p accumulated over key chunks (v_aug has a
    ones column -> row 64 = sum p).
  - PE warmup matmuls at t=0 engage the HAM clock gate (1.2 -> 2.4 GHz).
"""

import numpy as np
import ml_dtypes

import concourse.bass as bass
import concourse.mybir as mybir
import concourse.tile as tile
from concourse import bacc
from concourse.bass_utils import run_bass_kernel_spmd
from concourse.masks import make_identity

B, S, DM, DK = 2, 4096, 1024, 64
N_CORES = 8
SQ = 2048            # queries per core
SK = 2048            # keys per core
NG = 8               # key groups per core (256 keys each)
NJ = DM // 128       # dm chunks (8)

F32 = mybir.dt.float32
BF16 = mybir.dt.bfloat16
FP8 = mybir.dt.float8e4
DR = mybir.MatmulPerfMode.DoubleRow
EXP = mybir.ActivationFunctionType.Exp

MASK_W = 240.0       # ident weight: exp(0.125*(s + 240*m) - 30) = exp(s/8 + 30m - 30)
N_WARM = 20          # PE warmup matmuls
DEBUG = False        # add intermediate dumps

_last_results = None


def _build():
    nc = bacc.Bacc(None, target_bir_lowering=False)

    qt_e = nc.declare_dram_parameter("qt", [128, 2 * NJ * 1024], BF16, isOutput=False)
    kt_e = nc.declare_dram_parameter("kt", [128, NG * NJ * 256], BF16, isOutput=False)
    vt_e = nc.declare_dram_parameter("vt", [128, NG * NJ * 256], BF16, isOutput=False)
    m0_e = nc.declare_dram_parameter("m0", [128, 16, 1024], FP8, isOutput=False)
    m1_e = nc.declare_dram_parameter("m1", [128, 16, 1024], FP8, isOutput=False)
    w_e = nc.declare_dram_parameter("wqkv", [128, 3 * NJ * DK], BF16, isOutput=False)
    b_e = nc.declare_dram_parameter("bqkv", [128, 3], F32, isOutput=False)
    id_e = nc.declare_dram_parameter("identdr", [128, 2, 256], FP8, isOutput=False)
    out_e = nc.declare_dram_parameter("out", [65, SQ], F32, isOutput=True)
    if DEBUG:
        dbg_e = {
            "d_qT0": nc.declare_dram_parameter("d_qT0", [128, 1024], BF16, isOutput=True),
            "d_kT": nc.declare_dram_parameter("d_kT", [128, NG * 128], BF16, isOutput=True),
            "d_vaug": nc.declare_dram_parameter("d_vaug", [128, 16 * 65], BF16, isOutput=True),
            "d_p": nc.declare_dram_parameter("d_p", [128, 1024], BF16, isOutput=True),
        }

    with tile.TileContext(nc) as tc:
        with (
            tc.tile_pool(name="const", bufs=1) as cpool,
            tc.tile_pool(name="inp", bufs=1) as ipool,
            tc.tile_pool(name="work", bufs=1) as spool,
            tc.tile_pool(name="pp", bufs=3) as ppool,
            tc.tile_pool(name="ps_work", bufs=2, space="PSUM") as pwork,
            tc.tile_pool(name="ps_y", bufs=1, space="PSUM") as py,
            tc.tile_pool(name="ps_kv", bufs=1, space="PSUM") as pkv,
        ):
            # ---- constants / warmup (no DMA deps) ----
            wu = cpool.tile([128, 512], BF16, tag="wu")
            nc.vector.memset(wu[:], 0.0)
            nbias = cpool.tile([128, 1], F32, tag="nbias")
            nc.vector.memset(nbias[:], -30.0)
            act_w = spool.tile([128, 32], BF16, tag="actw")
            nc.scalar.activation(act_w[:], wu[:, 0:32], EXP, bias=nbias[:])  # pull exp tables early
            ident_bf = cpool.tile([128, 128], BF16, tag="identbf")
            make_identity(nc, ident_bf[:])

            wups = pwork.tile([128, 1024], F32, tag="sAB", name="wups")
            for i in range(N_WARM):
                nc.tensor.matmul(
                    wups[:, 0:512], lhsT=wu[:, 0:128], rhs=wu[:],
                    start=True, stop=True, skip_group_check=True,
                )

            # ---- input DMAs (issue order ~= arrival order) ----
            qt_sb = ipool.tile([128, 2 * NJ * 1024], BF16, tag="qt")
            nc.sync.dma_start(qt_sb[:, 0:8192], qt_e[:, 0:8192])
            w_sb = cpool.tile([128, 3 * NJ * DK], BF16, tag="w")
            nc.sync.dma_start(w_sb[:], w_e[:])
            b_sb = cpool.tile([128, 3], F32, tag="b")
            nc.sync.dma_start(b_sb[:], b_e[:])
            id_sb = cpool.tile([128, 2, 256], FP8, tag="ident")
            nc.sync.dma_start(id_sb[:], id_e[:])

            kt_sb = ipool.tile([128, NG * 2048], BF16, tag="kt")
            vt_sb = ipool.tile([128, NG * 2048], BF16, tag="vt")
            m0_sb = ipool.tile([128, 16, 1024], FP8, tag="m0")
            m1_sb = ipool.tile([128, 16, 1024], FP8, tag="m1")
            for g in range(NG):
                cs = slice(g * 2048, (g + 1) * 2048)
                nc.sync.dma_start(kt_sb[:, cs], kt_e[:, cs])
                nc.sync.dma_start(vt_sb[:, cs], vt_e[:, cs])
                nc.sync.dma_start(m0_sb[:, 2 * g:2 * g + 2, :], m0_e[:, 2 * g:2 * g + 2, :])
            nc.sync.dma_start(qt_sb[:, 8192:16384], qt_e[:, 8192:16384])
            nc.sync.dma_start(m1_sb[:], m1_e[:])

            def wsl(which, j):
                return w_sb[:, (which * NJ + j) * DK:(which * NJ + j + 1) * DK]

            # ---- persistent work tiles ----
            qT = {}
            kT = spool.tile([128, NG * 128], BF16, tag="kT")
            vT = spool.tile([64, NG * 256], BF16, tag="vT")
            v_aug = spool.tile([128, 16 * 65], BF16, tag="vaug")
            nc.vector.memset(v_aug[:], 1.0)

            def q_proj(h):
                qps = pwork.tile([128, 1024], F32, tag="sAB", name=f"qps{h}")
                for j in range(NJ):
                    for s in range(2):
                        rhs = qt_sb[:, h * 8192 + j * 1024 + s * 512:
                                    h * 8192 + j * 1024 + (s + 1) * 512]
                        for st in range(2):
                            nc.tensor.matmul(
                                qps[st * 64:(st + 1) * 64, s * 512:(s + 1) * 512],
                                lhsT=wsl(0, j), rhs=rhs,
                                start=(j == 0), stop=(j == NJ - 1),
                            )
                qT[h] = spool.tile([128, 1024], BF16, tag=f"qT{h}", name=f"qT{h}")
                nc.vector.tensor_scalar_add(qT[h][:], qps[:], b_sb[:, 0:1])

            def kv_proj(g):
                kps = pkv.tile([128, 128], F32, tag="kps", name=f"kps{g}")
                for j in range(NJ):
                    c0 = g * 2048 + j * 256
                    nc.tensor.matmul(
                        kps[0:64, :], lhsT=wsl(1, j), rhs=kt_sb[:, c0:c0 + 128],
                        start=(j == 0), stop=(j == NJ - 1),
                    )
                    nc.tensor.matmul(
                        kps[64:128, :], lhsT=wsl(1, j), rhs=kt_sb[:, c0 + 128:c0 + 256],
                        start=(j == 0), stop=(j == NJ - 1),
                    )
                nc.vector.tensor_scalar_add(
                    kT[:, g * 128:(g + 1) * 128], kps[:], b_sb[:, 1:2]
                )
                vps = pkv.tile([64, 256], F32, tag="vps", name=f"vps{g}")
                for j in range(NJ):
                    c0 = g * 2048 + j * 256
                    nc.tensor.matmul(
                        vps[:], lhsT=wsl(2, j), rhs=vt_sb[:, c0:c0 + 256],
                        start=(j == 0), stop=(j == NJ - 1),
                    )
                nc.vector.tensor_scalar_add(
                    vT[:, g * 256:(g + 1) * 256], vps[:], b_sb[0:64, 2:3]
                )
                for c in range(2):
                    vtr = pkv.tile([128, 64], BF16, tag="kps", name=f"vtr{g}_{c}")
                    nc.tensor.transpose(
                        vtr[:], vT[0:64, g * 256 + c * 128:g * 256 + (c + 1) * 128],
                        ident_bf[0:64, 0:64],
                    )
                    nc.vector.tensor_copy(
                        v_aug[:, (2 * g + c) * 65:(2 * g + c) * 65 + 64], vtr[:]
                    )

            def main_step(g, h, s, y_ps, m_sb):
                sAB = pwork.tile([128, 1024], F32, tag="sAB", name=f"s{h}_{g}_{s}")
                qc = slice(s * 512, (s + 1) * 512)
                kc = g * 128
                # scores: 4 concurrent (K=64, M=64) tiles
                nc.tensor.matmul(
                    sAB[0:64, 0:512], lhsT=kT[0:64, kc:kc + 64],
                    rhs=qT[h][0:64, qc], start=True, stop=False,
                    skip_group_check=True,
                )
                nc.tensor.matmul(
                    sAB[64:128, 0:512], lhsT=kT[0:64, kc + 64:kc + 128],
                    rhs=qT[h][0:64, qc], start=True, stop=False,
                    skip_group_check=True,
                )
                nc.tensor.matmul(
                    sAB[0:64, 512:1024], lhsT=kT[64:128, kc:kc + 64],
                    rhs=qT[h][64:128, qc], start=True, stop=False,
                    skip_group_check=True,
                )
                nc.tensor.matmul(
                    sAB[64:128, 512:1024], lhsT=kT[64:128, kc + 64:kc + 128],
                    rhs=qT[h][64:128, qc], start=True, stop=False,
                    skip_group_check=True,
                )
                # mask add: psum += 240*m via fp8 DoubleRow identity
                m_rhs = m_sb[:, 2 * g:2 * g + 2, s * 512:(s + 1) * 512]
                nc.tensor.matmul(
                    sAB[:, 0:512], lhsT=id_sb[:, :, 0:128], rhs=m_rhs,
                    start=False, stop=True, perf_mode=DR, skip_group_check=True,
                )
                nc.tensor.matmul(
                    sAB[:, 512:1024], lhsT=id_sb[:, :, 128:256], rhs=m_rhs,
                    start=False, stop=True, perf_mode=DR, skip_group_check=True,
                )
                p = ppool.tile([128, 1024], BF16, tag="p", name=f"p{h}_{g}_{s}")
                nc.scalar.activation(p[:], sAB[:], EXP, bias=nbias[:], scale=0.125)
                if DEBUG and (g, h, s) == (0, 0, 0):
                    nc.sync.dma_start(dbg_e["d_p"][:], p[:])
                nc.tensor.matmul(
                    y_ps[:, qc], lhsT=v_aug[:, (2 * g) * 65:(2 * g) * 65 + 65],
                    rhs=p[:, 0:512], start=(g == 0), stop=False,
                    skip_group_check=True,
                )
                nc.tensor.matmul(
                    y_ps[:, qc], lhsT=v_aug[:, (2 * g + 1) * 65:(2 * g + 1) * 65 + 65],
                    rhs=p[:, 512:1024], start=False, stop=(g == NG - 1),
                    skip_group_check=True,
                )

            # ---- pass 0 (q half 0) with per-group projections ----
            with nc.named_scope("qproj0"):
                q_proj(0)
            y0 = py.tile([65, 1024], F32, tag="y", name="y0")
            for g in range(NG):
                with nc.named_scope(f"kv{g}"):
                    kv_proj(g)
                with nc.named_scope(f"p0g{g}"):
                    for s in range(2):
                        main_step(g, 0, s, y0, m0_sb)
            ysb0 = spool.tile([65, 1024], F32, tag="ysb0")
            nc.vector.tensor_copy(ysb0[:], y0[:])
            nc.sync.dma_start(out_e[:, 0:1024], ysb0[:])

            # ---- pass 1 (q half 1) ----
            with nc.named_scope("qproj1"):
                q_proj(1)
            y1 = py.tile([65, 1024], F32, tag="y", name="y1")
            for g in range(NG):
                with nc.named_scope(f"p1g{g}"):
                    for s in range(2):
                        main_step(g, 1, s, y1, m1_sb)
            ysb1 = spool.tile([65, 1024], F32, tag="ysb1")
            nc.vector.tensor_copy(ysb1[:], y1[:])
            nc.sync.dma_start(out_e[:, 1024:2048], ysb1[:])

            if DEBUG:
                nc.sync.dma_start(dbg_e["d_qT0"][:], qT[0][:])
                nc.sync.dma_start(dbg_e["d_kT"][:], kT[:])
                nc.sync.dma_start(dbg_e["d_vaug"][:], v_aug[:])

    nc.finalize()
    return nc


def _pack_x(x):
    """[2048 rows, 1024 dm] f32 -> qt layout [128, 2*8*1024] (h, j, q')."""
    t = x.T.reshape(NJ, 128, 2, 1024)          # [j, p, h, q']
    return np.ascontiguousarray(
        t.transpose(1, 2, 0, 3).reshape(128, -1)
    ).astype(ml_dtypes.bfloat16)


def _pack_kv(x):
    """[2048 keys, 1024 dm] f32 -> [128, 8*8*256] (g, j, r)."""
    t = x.T.reshape(NJ, 128, NG, 256)          # [j, p, g, r]
    return np.ascontiguousarray(
        t.transpose(1, 2, 0, 3).reshape(128, -1)
    ).astype(ml_dtypes.bfloat16)


def _pack_mask(mblk):
    """mask block [2048 q, 2048 k] int -> (m0, m1) each [128, 16, 1024] fp8.
    element (key = g*256 + j*128 + p, q = h*1024 + q') at m{h}[p, 2g+j, q']."""
    t = mblk.T.reshape(NG, 2, 128, 2, 1024)    # [g, j, p, h, q']
    t = t.transpose(2, 3, 0, 1, 4)             # [p, h, g, j, q']
    m = np.ascontiguousarray(t.reshape(128, 2, 16, 1024)).astype(ml_dtypes.float8_e4m3)
    return m[:, 0], m[:, 1]


def kernel(Q, K, V, mask, Wq, bq, Wk, bk, Wv, bv):
    global _last_results
    bf16 = ml_dtypes.bfloat16
    fp8 = ml_dtypes.float8_e4m3

    Q, K, V = (np.asarray(a, dtype=np.float32) for a in (Q, K, V))
    mask = np.asarray(mask)

    w_p = np.concatenate(
        [np.ascontiguousarray(
            W.T.reshape(NJ, 128, DK).transpose(1, 0, 2).reshape(128, NJ * DK)
         ).astype(bf16) for W in (Wq, Wk, Wv)],
        axis=1,
    )
    b_p = np.ascontiguousarray(
        np.stack([np.tile(np.asarray(b, np.float32), 2) for b in (bq, bk, bv)], axis=1)
    )
    ident = np.zeros((128, 2, 2, 128), dtype=np.float32)
    for p in range(128):
        ident[p, 0, 0, p] = MASK_W
        ident[p, 1, 1, p] = MASK_W
    ident = ident.reshape(128, 2, 256).astype(fp8)

    qt_c = {(b, qh): _pack_x(Q[b, qh * SQ:(qh + 1) * SQ]) for b in range(B) for qh in range(2)}
    kt_c = {(b, kh): _pack_kv(K[b, kh * SK:(kh + 1) * SK]) for b in range(B) for kh in range(2)}
    vt_c = {(b, kh): _pack_kv(V[b, kh * SK:(kh + 1) * SK]) for b in range(B) for kh in range(2)}

    in_maps = []
    for c in range(N_CORES):
        b, r = divmod(c, 4)
        qh, kh = divmod(r, 2)
        m0, m1 = _pack_mask(mask[b, qh * SQ:(qh + 1) * SQ, kh * SK:(kh + 1) * SK])
        in_maps.append({
            "qt": qt_c[(b, qh)], "kt": kt_c[(b, kh)], "vt": vt_c[(b, kh)],
            "m0": m0, "m1": m1,
            "wqkv": w_p, "bqkv": b_p, "identdr": ident,
        })

    nc = _build()
    res = run_bass_kernel_spmd(nc, in_maps, core_ids=list(range(N_CORES)))
    _last_results = res

    out = np.empty((B, S, DK), dtype=np.float32)
    for b in range(B):
        for qh in range(2):
            yA = res.results[b * 4 + qh * 2 + 0]["out"].astype(np.float64)
            yB = res.results[b * 4 + qh * 2 + 1]["out"].astype(np.float64)
            ysum = yA + yB
            y = ysum[:DK] / ysum[DK:DK + 1]
            out[b, qh * SQ:(qh + 1) * SQ, :] = y.T.astype(np.float32)
    return out
